# revision 1
# baseline (speedup 1.0000x reference)
"""Trainium2 Bass kernel for CIN layer:
    out[b,c,d] = sum_{h,m} W[c, h*M+m] * xk[b,h,d] * x0[b,m,d] + bias[c]

Shapes (hardcoded): x0 [512,40,64] f32, xk [512,128,64] f32,
W [128,5120] f32, b [128] f32 -> out [512,128,64] f32.

Strategy: data-parallel over batch B across 8 cores (64 batches/core).
Per core, columns are the 64*64=4096 (b,d) pairs. The 5120-long (h,m)
contraction is split into 40 chunks of 128 rows with a mixed-radix
partition layout: chunk (g, j) covers m in the 8-wide group g (5 groups)
x h in the 16-wide block j (8 blocks); partition p holds
(m = 8g + p//16, h = 16j + p%16). Then
  outer[p, col] = xkrep_j[p, col] * x0bc_g[p, col]  (DVE TT, bf16 2x)
  psum[q] += w3[g,j][p,c].T @ outer[:, q*512:...]   (PE, accum 40 chunks)
where xkrep_j (xk h-block replicated 8x along partitions) and x0bc_g
(x0 m-group rows replicated 16x) are produced host-side (pure layout,
no arithmetic): only 8 + 5 = 13 replicated tiles total, each reused
across the other loop axis - 3.2x less DMA than a full x0 broadcast.
W is host-gathered to match the chunk layout. Bias-add is fused into
the PSUM->SBUF eviction on ScalarE.
"""

import numpy as np
import ml_dtypes

B, M, H, D, C = 512, 40, 128, 64, 128
N_CORES = 8
BC = B // N_CORES          # 64 batches per core
COLS = BC * D              # 4096 (b,d) columns per core
NG = 8                     # PSUM groups
GW = COLS // NG            # 512 columns per group
MG = 8                     # m-values per chunk group
NMG = M // MG              # 5 m-groups
HB = 128 // MG             # 16 h-values per block
NHB = H // HB              # 8 h-blocks
NCHUNK = NMG * NHB         # 40 contraction chunks

_cache = {}


def _build(reps=1):
    import contextlib

    import concourse.bacc as bacc
    import concourse.mybir as mybir
    from concourse.tile import TileContext

    f32 = mybir.dt.float32
    bf16 = mybir.dt.bfloat16

    nc = bacc.Bacc("TRN2", debug=False, num_devices=N_CORES)

    xkr_d = nc.dram_tensor("xkrep_in", [NHB, 128, COLS], bf16, kind="ExternalInput")
    x0b_d = nc.dram_tensor("x0bc_in", [NMG, 128, COLS], bf16, kind="ExternalInput")
    w3_d = nc.dram_tensor("w3_in", [NCHUNK, 128, C], bf16, kind="ExternalInput")
    bias_d = nc.dram_tensor("bias_in", [C, 1], f32, kind="ExternalInput")
    out_d = nc.dram_tensor("out", [BC, C, D], f32, kind="ExternalOutput")

    with TileContext(nc) as tc:
        with (
            tc.tile_pool(name="const", bufs=1) as cpool,
            tc.tile_pool(name="work", bufs=6) as wpool,
            tc.tile_pool(name="outp", bufs=2) as opool,
            tc.tile_pool(name="psum", bufs=1, space="PSUM") as ppool,
        ):
            # ---- load constants / replicated operand tiles ----
            # Each tile is loaded as two half-column DMAs, all phase-0
            # halves first (in rough first-use order), so phase 0 of the
            # main loop can start after ~half the prologue bytes; Tile's
            # subtile dependency tracking lets the half-column TT reads
            # wait only on their half's DMA.
            HC = COLS // 2
            w3_sb = cpool.tile([128, NCHUNK * C], bf16)
            w3_ap = w3_d.ap().rearrange("k p c -> p k c")

            bias_sb = cpool.tile([128, 1], f32)
            nc.sync.dma_start(out=bias_sb, in_=bias_d.ap())

            xkreps = [None] * NHB
            x0bcs = [None] * NMG
            load_order = [("x", 0), ("0", 0), ("x", 1), ("x", 2), ("0", 1),
                          ("x", 3), ("x", 4), ("0", 2), ("x", 5), ("x", 6),
                          ("0", 3), ("x", 7), ("0", 4)]
            for kind, i in load_order:
                if kind == "x":
                    xkr = cpool.tile(
                        [128, COLS], bf16, name=f"xkr{i}", tag=f"xkr{i}"
                    )
                    xkreps[i] = xkr
                else:
                    x0b = cpool.tile(
                        [128, COLS], bf16, name=f"x0b{i}", tag=f"x0b{i}"
                    )
                    x0bcs[i] = x0b
            nc.sync.dma_start(
                out=w3_sb[:, : NCHUNK * C // 2], in_=w3_ap[:, : NCHUNK // 2, :]
            )
            for ph in range(2):
                for kind, i in load_order:
                    tile_, src = (
                        (xkreps[i], xkr_d.ap()[i])
                        if kind == "x"
                        else (x0bcs[i], x0b_d.ap()[i])
                    )
                    nc.sync.dma_start(
                        out=tile_[:, ph * HC:(ph + 1) * HC],
                        in_=src[:, ph * HC:(ph + 1) * HC],
                    )
                if ph == 0:
                    nc.sync.dma_start(
                        out=w3_sb[:, NCHUNK * C // 2:],
                        in_=w3_ap[:, NCHUNK // 2:, :],
                    )

            loop_ctx = (
                tc.For_i(
                    0, reps, 1,
                    hint_engines=(mybir.EngineType.PE,),
                    staggered_reset=True,
                )
                if reps > 1
                else contextlib.nullcontext()
            )
            with loop_ctx:
                psums = []
                for q in range(NG):
                    ps = ppool.tile([128, GW], f32, name=f"ps{q}", tag=f"ps{q}")
                    psums.append(ps)

                if reps == 1:
                    # Warm the PE's HAM clock-gate (~3.4us of sustained
                    # activity -> 2.4 GHz) with dummy matmuls on scratch
                    # data while the prologue DMAs are still in flight.
                    # Each real first-accumulation MM uses start=True, so
                    # whatever these leave in PSUM is discarded.
                    scratch = cpool.tile([128, GW], bf16)
                    nc.gpsimd.memset(scratch, 0.0)
                    for _ in range(16):
                        nc.tensor.matmul(
                            psums[0],
                            lhsT=scratch[:, :128],
                            rhs=scratch,
                            start=True,
                            stop=True,
                        )

                # ---- main loop: two column phases over the 40 chunks ----
                # Phase ph sweeps all 40 contraction chunks for columns
                # [ph*2048, (ph+1)*2048) into PSUM banks ph*4..ph*4+3, then
                # evicts those banks while the other phase computes - so
                # the eviction + store tail overlaps compute instead of
                # serializing at the end. Within a phase, MMs are issued
                # in groups of GK chunks, bank-major inside the group, so
                # the PE stays on one PSUM bank for GK consecutive
                # matmuls instead of cycling banks every MM (bank cycling
                # measurably degrades PE throughput).
                GK = 5
                NSLOT = GK + 2
                HCOL = COLS // 2
                out_ap = out_d.ap().rearrange("b c d -> c b d")
                bpg = BC // NG  # batches per bank
                for ph in range(2):
                    for k0 in range(0, NCHUNK, GK):
                        outers = []
                        for k in range(k0, k0 + GK):
                            g, j = divmod(k, NHB)
                            outer = wpool.tile(
                                [128, HCOL], bf16, name=f"outer{ph}_{k}",
                                tag=f"outer{k % NSLOT}", bufs=1,
                            )
                            nc.vector.tensor_mul(
                                outer,
                                xkreps[j][:, ph * HCOL:(ph + 1) * HCOL],
                                x0bcs[g][:, ph * HCOL:(ph + 1) * HCOL],
                            )
                            outers.append(outer)
                        for ql in range(NG // 2):
                            qb = ph * (NG // 2) + ql
                            for i, k in enumerate(range(k0, k0 + GK)):
                                nc.tensor.matmul(
                                    psums[qb],
                                    lhsT=w3_sb[:, k * C:(k + 1) * C],
                                    rhs=outers[i][:, ql * GW:(ql + 1) * GW],
                                    start=(k == 0),
                                    stop=(k == NCHUNK - 1),
                                )
                    # bias add + store for this phase's banks
                    for ql in range(NG // 2):
                        qb = ph * (NG // 2) + ql
                        out_sb = opool.tile(
                            [128, GW], f32, name=f"osb{qb}", tag="osb"
                        )
                        nc.scalar.activation(
                            out_sb,
                            psums[qb],
                            mybir.ActivationFunctionType.Identity,
                            bias=bias_sb[:, 0:1],
                            scale=1.0,
                        )
                        nc.sync.dma_start(
                            out=out_ap[:, qb * bpg:(qb + 1) * bpg, :], in_=out_sb
                        )

    nc.compile()
    return nc


def _prep_host(x0, xk, W, b):
    """Host-side layout prep (no arithmetic): shard, transpose, replicate."""
    part = np.arange(128)
    hh = (part % HB)[None, :] + HB * np.arange(NHB)[:, None]   # [NHB, 128]
    mm = (part // HB)[None, :] + MG * np.arange(NMG)[:, None]  # [NMG, 128]

    Wr = W.reshape(C, H, M)
    w3 = np.empty((NCHUNK, 128, C), ml_dtypes.bfloat16)
    for g in range(NMG):
        for j in range(NHB):
            w3[g * NHB + j] = Wr[:, hh[j], mm[g]].T.astype(ml_dtypes.bfloat16)
    bias = np.ascontiguousarray(b.reshape(C, 1)).astype(np.float32)

    in_maps = []
    for k in range(N_CORES):
        x0s = x0[k * BC:(k + 1) * BC]            # [BC, M, D]
        xks = xk[k * BC:(k + 1) * BC]            # [BC, H, D]
        xk2 = (
            np.ascontiguousarray(xks.transpose(1, 0, 2))
            .reshape(H, COLS)
            .astype(ml_dtypes.bfloat16)
        )
        x02 = (
            np.ascontiguousarray(x0s.transpose(1, 0, 2))
            .reshape(M, COLS)
            .astype(ml_dtypes.bfloat16)
        )
        in_maps.append(
            {
                "xkrep_in": np.ascontiguousarray(xk2[hh]),
                "x0bc_in": np.ascontiguousarray(x02[mm]),
                "w3_in": w3,
                "bias_in": bias,
            }
        )
    return in_maps


def _run(in_maps, **kwargs):
    from concourse import bass_utils

    if "nc" not in _cache:
        _cache["nc"] = _build()
    return bass_utils.run_bass_kernel_spmd(
        _cache["nc"], in_maps, core_ids=list(range(N_CORES)), **kwargs
    )


def kernel(x0, xk, W, b, _bench=[None]):
    x0 = np.asarray(x0, dtype=np.float32)
    xk = np.asarray(xk, dtype=np.float32)
    W = np.asarray(W, dtype=np.float32)
    b = np.asarray(b, dtype=np.float32)
    in_maps = _prep_host(x0, xk, W, b)
    res = _run(in_maps)
    _bench[0] = res
    out = np.concatenate([r["out"] for r in res.results], axis=0)
    return out.astype(np.float32, copy=False)



# revision 3
# speedup vs baseline: 34.9481x; 34.9481x over previous
"""Trainium2 Bass kernel for CIN layer:
    out[b,c,d] = sum_{h,m} W[c, h*M+m] * xk[b,h,d] * x0[b,m,d] + bias[c]

Shapes (hardcoded): x0 [512,40,64] f32, xk [512,128,64] f32,
W [128,5120] f32, b [128] f32 -> out [512,128,64] f32.

Strategy: data-parallel over batch B across 8 cores (64 batches/core).
Per core, columns are the 64*64=4096 (b,d) pairs. The 5120-long (h,m)
contraction is split into 40 chunks of 128 rows with a mixed-radix
partition layout: chunk (g, j) covers m in the 8-wide group g (5 groups)
x h in the 16-wide block j (8 blocks); partition p holds
(m = 8g + p//16, h = 16j + p%16). Then
  outer[p, col] = xkrep_j[p, col] * x0bc_g[p, col]  (DVE TT, bf16 2x)
  psum[q] += w3[g,j][p,c].T @ outer[:, q*512:...]   (PE, accum 40 chunks)
where xkrep_j (xk h-block replicated 8x along partitions) and x0bc_g
(x0 m-group rows replicated 16x) are produced host-side (pure layout,
no arithmetic): only 8 + 5 = 13 replicated tiles total, each reused
across the other loop axis - 3.2x less DMA than a full x0 broadcast.
W is host-gathered to match the chunk layout. Bias-add is fused into
the PSUM->SBUF eviction on ScalarE.
"""

import numpy as np
import ml_dtypes

B, M, H, D, C = 512, 40, 128, 64, 128
N_CORES = 8
BC = B // N_CORES          # 64 batches per core
COLS = BC * D              # 4096 (b,d) columns per core
NG = 8                     # PSUM groups
GW = COLS // NG            # 512 columns per group
MG = 8                     # m-values per chunk group
NMG = M // MG              # 5 m-groups
HB = 128 // MG             # 16 h-values per block
NHB = H // HB              # 8 h-blocks
NCHUNK = NMG * NHB         # 40 contraction chunks

_cache = {}


def _build(reps=1):
    import contextlib

    import concourse.bacc as bacc
    import concourse.mybir as mybir
    from concourse.tile import TileContext

    f32 = mybir.dt.float32
    bf16 = mybir.dt.bfloat16

    nc = bacc.Bacc("TRN2", debug=False, num_devices=N_CORES)

    xkr_d = nc.dram_tensor("xkrep_in", [NHB, 128, COLS], bf16, kind="ExternalInput")
    x0b_d = nc.dram_tensor("x0bc_in", [NMG, 128, COLS], bf16, kind="ExternalInput")
    w3_d = nc.dram_tensor("w3_in", [NCHUNK, 128, C], bf16, kind="ExternalInput")
    bias_d = nc.dram_tensor("bias_in", [C, 1], f32, kind="ExternalInput")
    out_d = nc.dram_tensor("out", [BC, C, D], f32, kind="ExternalOutput")

    with TileContext(nc) as tc:
        with (
            tc.tile_pool(name="const", bufs=1) as cpool,
            tc.tile_pool(name="work", bufs=6) as wpool,
            tc.tile_pool(name="outp", bufs=2) as opool,
            tc.tile_pool(name="psum", bufs=1, space="PSUM") as ppool,
        ):
            # ---- load constants / replicated operand tiles ----
            # Each tile is loaded as two half-column DMAs, all phase-0
            # halves first (in rough first-use order), so phase 0 of the
            # main loop can start after ~half the prologue bytes; Tile's
            # subtile dependency tracking lets the half-column TT reads
            # wait only on their half's DMA.
            HC = COLS // 2
            w3_sb = cpool.tile([128, NCHUNK * C], bf16)
            w3_ap = w3_d.ap().rearrange("k p c -> p k c")

            bias_sb = cpool.tile([128, 1], f32)
            nc.sync.dma_start(out=bias_sb, in_=bias_d.ap())

            xkreps = [None] * NHB
            x0bcs = [None] * NMG
            load_order = [("x", 0), ("0", 0), ("x", 1), ("x", 2), ("0", 1),
                          ("x", 3), ("x", 4), ("0", 2), ("x", 5), ("x", 6),
                          ("0", 3), ("x", 7), ("0", 4)]
            for kind, i in load_order:
                if kind == "x":
                    xkr = cpool.tile(
                        [128, COLS], bf16, name=f"xkr{i}", tag=f"xkr{i}"
                    )
                    xkreps[i] = xkr
                else:
                    x0b = cpool.tile(
                        [128, COLS], bf16, name=f"x0b{i}", tag=f"x0b{i}"
                    )
                    x0bcs[i] = x0b
            nc.sync.dma_start(
                out=w3_sb[:, : NCHUNK * C // 2], in_=w3_ap[:, : NCHUNK // 2, :]
            )
            for ph in range(2):
                for kind, i in load_order:
                    tile_, src = (
                        (xkreps[i], xkr_d.ap()[i])
                        if kind == "x"
                        else (x0bcs[i], x0b_d.ap()[i])
                    )
                    nc.sync.dma_start(
                        out=tile_[:, ph * HC:(ph + 1) * HC],
                        in_=src[:, ph * HC:(ph + 1) * HC],
                    )
                if ph == 0:
                    nc.sync.dma_start(
                        out=w3_sb[:, NCHUNK * C // 2:],
                        in_=w3_ap[:, NCHUNK // 2:, :],
                    )

            loop_ctx = (
                tc.For_i(
                    0, reps, 1,
                    hint_engines=(mybir.EngineType.PE,),
                    staggered_reset=True,
                )
                if reps > 1
                else contextlib.nullcontext()
            )
            with loop_ctx:
                psums = []
                for q in range(NG):
                    ps = ppool.tile([128, GW], f32, name=f"ps{q}", tag=f"ps{q}")
                    psums.append(ps)

                if reps == 1:
                    # Warm the PE's HAM clock-gate (~3.4us of sustained
                    # activity -> 2.4 GHz) with dummy matmuls on scratch
                    # data while the prologue DMAs are still in flight.
                    # Each real first-accumulation MM uses start=True, so
                    # whatever these leave in PSUM is discarded.
                    scratch = cpool.tile([128, GW], bf16)
                    nc.vector.memset(scratch, 0.0)
                    for _ in range(16):
                        nc.tensor.matmul(
                            psums[0],
                            lhsT=scratch[:, :128],
                            rhs=scratch,
                            start=True,
                            stop=True,
                        )

                # ---- main loop: two column phases over the 40 chunks ----
                # Phase ph sweeps all 40 contraction chunks for columns
                # [ph*2048, (ph+1)*2048) into PSUM banks ph*4..ph*4+3, then
                # evicts those banks while the other phase computes - so
                # the eviction + store tail overlaps compute instead of
                # serializing at the end. Within a phase, MMs are issued
                # in groups of GK chunks, bank-major inside the group, so
                # the PE stays on one PSUM bank for GK consecutive
                # matmuls instead of cycling banks every MM (bank cycling
                # measurably degrades PE throughput).
                # The 21M-elem outer-product stream is the kernel's real
                # bottleneck (DVE TT bf16 caps at 2 elem/lane/cycle =
                # ~1.22us per [128,2048] chunk; 80 chunks = ~98us vs the
                # PE's ~68us of matmul).  Offload the last chunk of every
                # 5-chunk group to GpSimd (~4.7us per chunk there): DVE
                # 64 chunks ~78us -> 32 chunks/phase = ~39us/phase, GpSimd
                # 8 chunks/phase ~38us/phase, PE ~34us/phase - all three
                # engines near-balanced.
                GK = 5
                ND_SLOT = 8
                NG_SLOT = 3
                HCOL = COLS // 2
                out_ap = out_d.ap().rearrange("b c d -> c b d")
                bpg = BC // NG  # batches per bank
                for ph in range(2):
                    ndve = 0
                    for k0 in range(0, NCHUNK, GK):
                        outers = []
                        for k in range(k0, k0 + GK):
                            g, j = divmod(k, NHB)
                            if k % GK == GK - 1:
                                outer = wpool.tile(
                                    [128, HCOL], bf16, name=f"outg{ph}_{k}",
                                    tag=f"og{(k // GK) % NG_SLOT}", bufs=1,
                                )
                                eng = nc.gpsimd
                            else:
                                outer = wpool.tile(
                                    [128, HCOL], bf16, name=f"outd{ph}_{k}",
                                    tag=f"od{ndve % ND_SLOT}", bufs=1,
                                )
                                eng = nc.vector
                                ndve += 1
                            eng.tensor_mul(
                                outer,
                                xkreps[j][:, ph * HCOL:(ph + 1) * HCOL],
                                x0bcs[g][:, ph * HCOL:(ph + 1) * HCOL],
                            )
                            outers.append(outer)
                        for ql in range(NG // 2):
                            qb = ph * (NG // 2) + ql
                            for i, k in enumerate(range(k0, k0 + GK)):
                                nc.tensor.matmul(
                                    psums[qb],
                                    lhsT=w3_sb[:, k * C:(k + 1) * C],
                                    rhs=outers[i][:, ql * GW:(ql + 1) * GW],
                                    start=(k == 0),
                                    stop=(k == NCHUNK - 1),
                                )
                    # bias add + store for this phase's banks
                    for ql in range(NG // 2):
                        qb = ph * (NG // 2) + ql
                        out_sb = opool.tile(
                            [128, GW], f32, name=f"osb{qb}", tag="osb"
                        )
                        nc.scalar.activation(
                            out_sb,
                            psums[qb],
                            mybir.ActivationFunctionType.Identity,
                            bias=bias_sb[:, 0:1],
                            scale=1.0,
                        )
                        nc.sync.dma_start(
                            out=out_ap[:, qb * bpg:(qb + 1) * bpg, :], in_=out_sb
                        )

    nc.compile()
    return nc


def _prep_host(x0, xk, W, b):
    """Host-side layout prep (no arithmetic): shard, transpose, replicate."""
    part = np.arange(128)
    hh = (part % HB)[None, :] + HB * np.arange(NHB)[:, None]   # [NHB, 128]
    mm = (part // HB)[None, :] + MG * np.arange(NMG)[:, None]  # [NMG, 128]

    Wr = W.reshape(C, H, M)
    w3 = np.empty((NCHUNK, 128, C), ml_dtypes.bfloat16)
    for g in range(NMG):
        for j in range(NHB):
            w3[g * NHB + j] = Wr[:, hh[j], mm[g]].T.astype(ml_dtypes.bfloat16)
    bias = np.ascontiguousarray(b.reshape(C, 1)).astype(np.float32)

    in_maps = []
    for k in range(N_CORES):
        x0s = x0[k * BC:(k + 1) * BC]            # [BC, M, D]
        xks = xk[k * BC:(k + 1) * BC]            # [BC, H, D]
        xk2 = (
            np.ascontiguousarray(xks.transpose(1, 0, 2))
            .reshape(H, COLS)
            .astype(ml_dtypes.bfloat16)
        )
        x02 = (
            np.ascontiguousarray(x0s.transpose(1, 0, 2))
            .reshape(M, COLS)
            .astype(ml_dtypes.bfloat16)
        )
        in_maps.append(
            {
                "xkrep_in": np.ascontiguousarray(xk2[hh]),
                "x0bc_in": np.ascontiguousarray(x02[mm]),
                "w3_in": w3,
                "bias_in": bias,
            }
        )
    return in_maps


def _run(in_maps, **kwargs):
    from concourse import bass_utils

    if "nc" not in _cache:
        _cache["nc"] = _build()
    return bass_utils.run_bass_kernel_spmd(
        _cache["nc"], in_maps, core_ids=list(range(N_CORES)), **kwargs
    )


def kernel(x0, xk, W, b, _bench=[None]):
    x0 = np.asarray(x0, dtype=np.float32)
    xk = np.asarray(xk, dtype=np.float32)
    W = np.asarray(W, dtype=np.float32)
    b = np.asarray(b, dtype=np.float32)
    in_maps = _prep_host(x0, xk, W, b)
    res = _run(in_maps)
    _bench[0] = res
    out = np.concatenate([r["out"] for r in res.results], axis=0)
    return out.astype(np.float32, copy=False)



# revision 7
# speedup vs baseline: 44.9632x; 1.2866x over previous
"""Trainium2 Bass kernel for CIN layer:
    out[b,c,d] = sum_{h,m} W[c, h*M+m] * xk[b,h,d] * x0[b,m,d] + bias[c]

Shapes (hardcoded): x0 [512,40,64] f32, xk [512,128,64] f32,
W [128,5120] f32, b [128] f32 -> out [512,128,64] f32.

Strategy: data-parallel over batch B across 8 cores (64 batches/core).
Per core, columns are the 64*64=4096 (b,d) pairs. The 5120-long (h,m)
contraction is split into 40 chunks of 128 rows with a mixed-radix
partition layout: chunk (g, j) covers m in the 8-wide group g (5 groups)
x h in the 16-wide block j (8 blocks); partition p holds
(m = 8g + p//16, h = 16j + p%16). Then
  outer[p, col] = xkrep_j[p, col] * x0bc_g[p, col]  (elementwise, bf16)
  psum[q] += w3[k][p,c].T @ outer[:, q*512:...]     (PE, accum 40 chunks)
xkrep_j / x0bc_g are produced host-side (pure layout, no arithmetic).

The elementwise outer-product stream (21M elem/core) exceeds what the
DVE alone can sustain (bf16 tensor_tensor caps at 2 elem/lane/cycle),
so the last chunk of each 5-chunk group is produced on GpSimd instead.
GpSimd chunks are consumed one group LATE (lag-1) so the slower engine
(~4.2us/chunk vs DVE ~1.1us) is never on the PE's critical path.

Columns are processed in two phases of 2048 so the PSUM eviction (bias
add on ScalarE + store) of phase 0 overlaps phase 1 compute.  The first
group's DVE chunks and their input DMAs are split into 1024-col quarters
so the first matmul can start ~2.5us in instead of waiting for full
half-tiles.  W is stored pre-transposed [128, 40*128] and the output
DRAM tensor is c-major [C, BC, D] so every DMA descriptor moves >=2KB
contiguous (sub-512B descriptors pay a 2x DMA-time penalty).
"""

import numpy as np
import ml_dtypes

B, M, H, D, C = 512, 40, 128, 64, 128
N_CORES = 8
BC = B // N_CORES          # 64 batches per core
COLS = BC * D              # 4096 (b,d) columns per core
NG = 8                     # PSUM groups
GW = COLS // NG            # 512 columns per group
MG = 8                     # m-values per chunk group
NMG = M // MG              # 5 m-groups
HB = 128 // MG             # 16 h-values per block
NHB = H // HB              # 8 h-blocks
NCHUNK = NMG * NHB         # 40 contraction chunks

_cache = {}


def _build(reps=1):
    import contextlib

    import concourse.bacc as bacc
    import concourse.mybir as mybir
    from concourse.tile import TileContext

    f32 = mybir.dt.float32
    bf16 = mybir.dt.bfloat16

    nc = bacc.Bacc("TRN2", debug=False, num_devices=N_CORES)

    xkr_d = nc.dram_tensor("xkrep_in", [NHB, 128, COLS], bf16, kind="ExternalInput")
    x0b_d = nc.dram_tensor("x0bc_in", [NMG, 128, COLS], bf16, kind="ExternalInput")
    # pre-transposed: partition-major, 10KB contiguous per partition row
    w3_d = nc.dram_tensor("w3_in", [128, NCHUNK * C], bf16, kind="ExternalInput")
    bias_d = nc.dram_tensor("bias_in", [C, 1], f32, kind="ExternalInput")
    # c-major so each output descriptor is a contiguous 2KB (b,d) run
    out_d = nc.dram_tensor("out", [C, BC, D], f32, kind="ExternalOutput")

    GK = 5
    NGRP = NCHUNK // GK        # 8 groups per phase
    ND_SLOT = 8
    NG_SLOT = 3
    HCOL = COLS // 2
    QCOL = COLS // 4

    with TileContext(nc) as tc:
        with (
            tc.tile_pool(name="const", bufs=1) as cpool,
            tc.tile_pool(name="work", bufs=6) as wpool,
            tc.tile_pool(name="outp", bufs=2) as opool,
            tc.tile_pool(name="psum", bufs=1, space="PSUM") as ppool,
        ):
            # ---- SBUF constant tiles ----
            w3_sb = cpool.tile([128, NCHUNK * C], bf16)
            bias_sb = cpool.tile([128, 1], f32)
            xkreps = [
                cpool.tile([128, COLS], bf16, name=f"xkr{i}", tag=f"xkr{i}")
                for i in range(NHB)
            ]
            x0bcs = [
                cpool.tile([128, COLS], bf16, name=f"x0b{i}", tag=f"x0b{i}")
                for i in range(NMG)
            ]

            # ---- prologue DMA, first-use order ----
            # w3 quarter for chunks 0-9 first (first MM operand), then the
            # group-0 operand tiles at quarter-column granularity so the
            # first TT/MM chain starts as early as possible.
            def load_tile(kind, i, c0, c1):
                t = xkreps[i] if kind == "x" else x0bcs[i]
                src = (xkr_d if kind == "x" else x0b_d).ap()[i]
                nc.sync.dma_start(out=t[:, c0:c1], in_=src[:, c0:c1])

            nc.sync.dma_start(
                out=w3_sb[:, : 10 * C], in_=w3_d.ap()[:, : 10 * C]
            )
            boot = [("x", 0), ("0", 0), ("x", 1), ("x", 2), ("x", 3)]
            for kind, i in boot:
                load_tile(kind, i, 0, QCOL)
            for kind, i in boot:
                load_tile(kind, i, QCOL, HCOL)
            load_tile("x", 4, 0, HCOL)
            nc.sync.dma_start(
                out=w3_sb[:, 10 * C: 20 * C], in_=w3_d.ap()[:, 10 * C: 20 * C]
            )
            # remaining phase-0 halves in first-use order
            rest = [("x", 5), ("x", 6), ("x", 7),
                    ("0", 1), ("0", 2), ("0", 3), ("0", 4)]
            for kind, i in rest[:3]:
                load_tile(kind, i, 0, HCOL)
            nc.sync.dma_start(
                out=w3_sb[:, 20 * C:], in_=w3_d.ap()[:, 20 * C:]
            )
            for kind, i in rest[3:]:
                load_tile(kind, i, 0, HCOL)
            # phase-1 halves, first-use order
            order = [("x", 0), ("0", 0), ("x", 1), ("x", 2), ("x", 3),
                     ("x", 4)] + rest
            for kind, i in order:
                load_tile(kind, i, HCOL, COLS)
            nc.sync.dma_start(out=bias_sb, in_=bias_d.ap())

            loop_ctx = (
                tc.For_i(
                    0, reps, 1,
                    hint_engines=(mybir.EngineType.PE,),
                    staggered_reset=True,
                )
                if reps > 1
                else contextlib.nullcontext()
            )
            with loop_ctx:
                psums = []
                for q in range(NG):
                    ps = ppool.tile([128, GW], f32, name=f"ps{q}", tag=f"ps{q}")
                    psums.append(ps)

                if reps == 1:
                    # Warm the PE (HAM clock-gate needs ~3.4us of sustained
                    # activity to reach 2.4 GHz) with dummy matmuls while
                    # the first operand DMAs are in flight.  start=True on
                    # each real first-accumulation MM discards the garbage.
                    scratch = cpool.tile([128, GW], bf16)
                    nc.gpsimd.memset(scratch, 0.0)
                    for _ in range(12):
                        nc.tensor.matmul(
                            psums[0],
                            lhsT=scratch[:, :128],
                            rhs=scratch,
                            start=True,
                            stop=True,
                        )

                # ---- main loop: two column phases over 8 chunk groups ----
                # Group i's MM block consumes its own 4 DVE chunks plus the
                # GpSimd chunk of group i-1 (lag-1); the last group also
                # consumes its own 5th (DVE) chunk and carries stop=True.
                for ph in range(2):
                    ndve = 0
                    pending_gp = None  # (k, tile) produced by GpSimd
                    for gi in range(NGRP):
                        k0 = gi * GK
                        mm_list = []        # (k, tile, col_off) in MM order
                        if pending_gp is not None:
                            mm_list.append(pending_gp)
                        last_grp = gi == NGRP - 1
                        dve_ks = list(range(k0, k0 + GK)) if last_grp else \
                            list(range(k0, k0 + GK - 1))
                        for k in dve_ks:
                            g, j = divmod(k, NHB)
                            outer = wpool.tile(
                                [128, HCOL], bf16, name=f"outd{ph}_{k}",
                                tag=f"od{ndve % ND_SLOT}", bufs=1,
                            )
                            ndve += 1
                            if ph == 0 and gi == 0:
                                # bootstrap: quarter-width TTs so the first
                                # MMs only wait on quarter DMAs
                                for q0 in (0, QCOL):
                                    nc.vector.tensor_mul(
                                        outer[:, q0:q0 + QCOL],
                                        xkreps[j][:, q0:q0 + QCOL],
                                        x0bcs[g][:, q0:q0 + QCOL],
                                    )
                            else:
                                nc.vector.tensor_mul(
                                    outer,
                                    xkreps[j][:, ph * HCOL:(ph + 1) * HCOL],
                                    x0bcs[g][:, ph * HCOL:(ph + 1) * HCOL],
                                )
                            mm_list.append((k, outer))
                        if not last_grp:
                            kg = k0 + GK - 1
                            g, j = divmod(kg, NHB)
                            gouter = wpool.tile(
                                [128, HCOL], bf16, name=f"outg{ph}_{kg}",
                                tag=f"og{gi % NG_SLOT}", bufs=1,
                            )
                            nc.gpsimd.tensor_mul(
                                gouter,
                                xkreps[j][:, ph * HCOL:(ph + 1) * HCOL],
                                x0bcs[g][:, ph * HCOL:(ph + 1) * HCOL],
                            )
                            pending_gp = (kg, gouter)
                        for ql in range(NG // 2):
                            qb = ph * (NG // 2) + ql
                            for k, outer in mm_list:
                                nc.tensor.matmul(
                                    psums[qb],
                                    lhsT=w3_sb[:, k * C:(k + 1) * C],
                                    rhs=outer[:, ql * GW:(ql + 1) * GW],
                                    start=(k == 0),
                                    stop=(last_grp and k == NCHUNK - 1),
                                )
                    # bias add + store for this phase's banks (bank-major MM
                    # order staggers bank completion, so earlier banks'
                    # eviction+store overlap the last group's matmuls)
                    for ql in range(NG // 2):
                        qb = ph * (NG // 2) + ql
                        out_sb = opool.tile(
                            [128, GW], f32, name=f"osb{qb}", tag="osb"
                        )
                        nc.scalar.activation(
                            out_sb,
                            psums[qb],
                            mybir.ActivationFunctionType.Identity,
                            bias=bias_sb[:, 0:1],
                            scale=1.0,
                        )
                        nc.sync.dma_start(
                            out=out_d.ap()[:, qb * (BC // NG):(qb + 1) * (BC // NG), :],
                            in_=out_sb,
                        )

    nc.compile()
    return nc


def _prep_host(x0, xk, W, b):
    """Host-side layout prep (no arithmetic): shard, transpose, replicate."""
    part = np.arange(128)
    hh = (part % HB)[None, :] + HB * np.arange(NHB)[:, None]   # [NHB, 128]
    mm = (part // HB)[None, :] + MG * np.arange(NMG)[:, None]  # [NMG, 128]

    Wr = W.reshape(C, H, M)
    w3 = np.empty((128, NCHUNK, C), ml_dtypes.bfloat16)
    for g in range(NMG):
        for j in range(NHB):
            w3[:, g * NHB + j, :] = Wr[:, hh[j], mm[g]].T.astype(
                ml_dtypes.bfloat16
            )
    w3 = np.ascontiguousarray(w3.reshape(128, NCHUNK * C))
    bias = np.ascontiguousarray(b.reshape(C, 1)).astype(np.float32)

    in_maps = []
    for k in range(N_CORES):
        x0s = x0[k * BC:(k + 1) * BC]            # [BC, M, D]
        xks = xk[k * BC:(k + 1) * BC]            # [BC, H, D]
        xk2 = (
            np.ascontiguousarray(xks.transpose(1, 0, 2))
            .reshape(H, COLS)
            .astype(ml_dtypes.bfloat16)
        )
        x02 = (
            np.ascontiguousarray(x0s.transpose(1, 0, 2))
            .reshape(M, COLS)
            .astype(ml_dtypes.bfloat16)
        )
        in_maps.append(
            {
                "xkrep_in": np.ascontiguousarray(xk2[hh]),
                "x0bc_in": np.ascontiguousarray(x02[mm]),
                "w3_in": w3,
                "bias_in": bias,
            }
        )
    return in_maps


def _run(in_maps, **kwargs):
    from concourse import bass_utils

    if "nc" not in _cache:
        _cache["nc"] = _build()
    return bass_utils.run_bass_kernel_spmd(
        _cache["nc"], in_maps, core_ids=list(range(N_CORES)), **kwargs
    )


def kernel(x0, xk, W, b, _bench=[None]):
    x0 = np.asarray(x0, dtype=np.float32)
    xk = np.asarray(xk, dtype=np.float32)
    W = np.asarray(W, dtype=np.float32)
    b = np.asarray(b, dtype=np.float32)
    in_maps = _prep_host(x0, xk, W, b)
    res = _run(in_maps)
    _bench[0] = res
    # per-core out is c-major [C, BC, D]; restore [BC, C, D] and stack cores
    out = np.concatenate(
        [np.transpose(r["out"], (1, 0, 2)) for r in res.results], axis=0
    )
    return np.ascontiguousarray(out, dtype=np.float32)


# revision 10
# speedup vs baseline: 47.5831x; 1.0583x over previous
"""Trainium2 Bass kernel for CIN layer:
    out[b,c,d] = sum_{h,m} W[c, h*M+m] * xk[b,h,d] * x0[b,m,d] + bias[c]

Shapes (hardcoded): x0 [512,40,64] f32, xk [512,128,64] f32,
W [128,5120] f32, b [128] f32 -> out [512,128,64] f32.

Strategy: data-parallel over batch B across 8 cores (64 batches/core).
Per core, columns are the 64*64=4096 (b,d) pairs. The 5120-long (h,m)
contraction is split into 40 chunks of 128 rows with a mixed-radix
partition layout: chunk (g, j) covers m in the 8-wide group g (5 groups)
x h in the 16-wide block j (8 blocks); partition p holds
(m = 8g + p//16, h = 16j + p%16). Then
  outer[p, col] = xkrep_j[p, col] * x0bc_g[p, col]  (elementwise, bf16)
  psum[q] += w3[k][p,c].T @ outer[:, q*512:...]     (PE, accum 40 chunks)
xkrep_j / x0bc_g are produced host-side (pure layout, no arithmetic).

The elementwise outer-product stream (21M elem/core) exceeds what the
DVE alone can sustain (bf16 tensor_tensor caps at 2 elem/lane/cycle),
so the last chunk of each 5-chunk group is produced on GpSimd instead.
GpSimd chunks are consumed one group LATE (lag-1) so the slower engine
(~4.2us/chunk vs DVE ~1.1us) is never on the PE's critical path.

Columns are processed in two phases of 2048 so the PSUM eviction (bias
add on ScalarE + store) of phase 0 overlaps phase 1 compute.  The first
group's DVE chunks and their input DMAs are split into 1024-col quarters
so the first matmul can start ~2.5us in instead of waiting for full
half-tiles.  W is stored pre-transposed [128, 40*128] and the output
DRAM tensor is c-major [C, BC, D] so every DMA descriptor moves >=2KB
contiguous (sub-512B descriptors pay a 2x DMA-time penalty).
"""

import numpy as np
import ml_dtypes

B, M, H, D, C = 512, 40, 128, 64, 128
N_CORES = 8
BC = B // N_CORES          # 64 batches per core
COLS = BC * D              # 4096 (b,d) columns per core
NG = 8                     # PSUM groups
GW = COLS // NG            # 512 columns per group
MG = 8                     # m-values per chunk group
NMG = M // MG              # 5 m-groups
HB = 128 // MG             # 16 h-values per block
NHB = H // HB              # 8 h-blocks
NCHUNK = NMG * NHB         # 40 contraction chunks

_cache = {}


def _build(reps=1):
    import contextlib

    import concourse.bacc as bacc
    import concourse.mybir as mybir
    from concourse.tile import TileContext

    f32 = mybir.dt.float32
    bf16 = mybir.dt.bfloat16

    nc = bacc.Bacc("TRN2", debug=False, num_devices=N_CORES)

    xkr_d = nc.dram_tensor("xkrep_in", [NHB, 128, COLS], bf16, kind="ExternalInput")
    x0b_d = nc.dram_tensor("x0bc_in", [NMG, 128, COLS], bf16, kind="ExternalInput")
    # pre-transposed: partition-major, 10KB contiguous per partition row
    w3_d = nc.dram_tensor("w3_in", [128, NCHUNK * C], bf16, kind="ExternalInput")
    bias_d = nc.dram_tensor("bias_in", [C, 1], f32, kind="ExternalInput")
    # c-major so each output descriptor is a contiguous 2KB (b,d) run
    out_d = nc.dram_tensor("out", [C, BC, D], f32, kind="ExternalOutput")

    GK = 5
    NGRP = NCHUNK // GK        # 8 groups per phase
    ND_SLOT = 9
    NG_SLOT = 4
    HCOL = COLS // 2
    QCOL = COLS // 4

    with TileContext(nc) as tc:
        with (
            tc.tile_pool(name="const", bufs=1) as cpool,
            tc.tile_pool(name="work", bufs=6) as wpool,
            tc.tile_pool(name="outp", bufs=2) as opool,
            tc.tile_pool(name="psum", bufs=1, space="PSUM") as ppool,
        ):
            # ---- SBUF constant tiles ----
            w3_sb = cpool.tile([128, NCHUNK * C], bf16)
            bias_sb = cpool.tile([128, 1], f32)
            xkreps = [
                cpool.tile([128, COLS], bf16, name=f"xkr{i}", tag=f"xkr{i}")
                for i in range(NHB)
            ]
            x0bcs = [
                cpool.tile([128, COLS], bf16, name=f"x0b{i}", tag=f"x0b{i}")
                for i in range(NMG)
            ]

            # ---- prologue DMA, first-use order ----
            # Group-0 operand tiles at quarter-column granularity (with the
            # w3 chunk-0..9 slice third) so the first TT/MM chain starts
            # ~2.5us in; then everything else in first-use order.
            def load_tile(kind, i, c0, c1):
                t = xkreps[i] if kind == "x" else x0bcs[i]
                src = (xkr_d if kind == "x" else x0b_d).ap()[i]
                nc.sync.dma_start(out=t[:, c0:c1], in_=src[:, c0:c1])

            boot = [("x", 0), ("0", 0), ("x", 1), ("x", 2), ("x", 3)]
            for n, (kind, i) in enumerate(boot):
                load_tile(kind, i, 0, QCOL)
                if n == 1:
                    nc.sync.dma_start(
                        out=w3_sb[:, : 10 * C], in_=w3_d.ap()[:, : 10 * C]
                    )
            for kind, i in boot:
                load_tile(kind, i, QCOL, HCOL)
            load_tile("x", 4, 0, HCOL)
            nc.sync.dma_start(
                out=w3_sb[:, 10 * C: 20 * C], in_=w3_d.ap()[:, 10 * C: 20 * C]
            )
            # remaining phase-0 halves in first-use order
            rest = [("x", 5), ("x", 6), ("0", 1), ("x", 7),
                    ("0", 2), ("0", 3), ("0", 4)]
            for kind, i in rest[:4]:
                load_tile(kind, i, 0, HCOL)
            nc.sync.dma_start(
                out=w3_sb[:, 20 * C:], in_=w3_d.ap()[:, 20 * C:]
            )
            for kind, i in rest[4:]:
                load_tile(kind, i, 0, HCOL)
            # phase-1 halves, first-use order
            order = [("x", 0), ("0", 0), ("x", 1), ("x", 2), ("x", 3),
                     ("x", 4)] + rest
            for kind, i in order:
                load_tile(kind, i, HCOL, COLS)
            nc.sync.dma_start(out=bias_sb, in_=bias_d.ap())

            loop_ctx = (
                tc.For_i(
                    0, reps, 1,
                    hint_engines=(mybir.EngineType.PE,),
                    staggered_reset=True,
                )
                if reps > 1
                else contextlib.nullcontext()
            )
            with loop_ctx:
                psums = []
                for q in range(NG):
                    ps = ppool.tile([128, GW], f32, name=f"ps{q}", tag=f"ps{q}")
                    psums.append(ps)

                if reps == 1:
                    # Warm the PE (HAM clock-gate needs ~3.4us of sustained
                    # activity to reach 2.4 GHz) with dummy matmuls while
                    # the first operand DMAs are in flight.  start=True on
                    # each real first-accumulation MM discards the garbage.
                    scratch = cpool.tile([128, GW], bf16)
                    nc.gpsimd.memset(scratch, 0.0)
                    for _ in range(12):
                        nc.tensor.matmul(
                            psums[0],
                            lhsT=scratch[:, :128],
                            rhs=scratch,
                            start=True,
                            stop=True,
                        )

                # ---- main loop: two column phases over 8 chunk groups ----
                # GpSimd-produced chunks are consumed `lag` groups after
                # production so the slow engine never gates the PE.  In
                # phase 0 the lags are longer because GpSimd's inputs are
                # still arriving over DMA.
                # gp_sched[ph] = {chunk: (prod_group, cons_group)}
                gp_sched = [
                    {4: (0, 3), 14: (2, 4), 19: (3, 5), 24: (4, 6),
                     29: (5, 7), 34: (6, 7)},
                    {4: (0, 1), 9: (1, 2), 14: (2, 3), 19: (3, 4),
                     24: (4, 5), 29: (5, 6), 34: (6, 7)},
                ]
                for ph in range(2):
                    sched = gp_sched[ph]
                    ndve = 0
                    ngp = 0
                    pending = {}    # cons_group -> [(k, tile)]
                    for gi in range(NGRP):
                        k0 = gi * GK
                        last_grp = gi == NGRP - 1
                        mm_list = list(pending.pop(gi, []))
                        for k in range(k0, k0 + GK):
                            g, j = divmod(k, NHB)
                            if k in sched:
                                gouter = wpool.tile(
                                    [128, HCOL], bf16, name=f"outg{ph}_{k}",
                                    tag=f"og{ngp % NG_SLOT}", bufs=1,
                                )
                                ngp += 1
                                nc.gpsimd.tensor_mul(
                                    gouter,
                                    xkreps[j][:, ph * HCOL:(ph + 1) * HCOL],
                                    x0bcs[g][:, ph * HCOL:(ph + 1) * HCOL],
                                )
                                pending.setdefault(sched[k][1], []).append(
                                    (k, gouter)
                                )
                                continue
                            outer = wpool.tile(
                                [128, HCOL], bf16, name=f"outd{ph}_{k}",
                                tag=f"od{ndve % ND_SLOT}", bufs=1,
                            )
                            ndve += 1
                            if ph == 0 and gi == 0:
                                # bootstrap: quarter-width TTs so the first
                                # MMs only wait on quarter DMAs
                                for q0 in (0, QCOL):
                                    nc.vector.tensor_mul(
                                        outer[:, q0:q0 + QCOL],
                                        xkreps[j][:, q0:q0 + QCOL],
                                        x0bcs[g][:, q0:q0 + QCOL],
                                    )
                            else:
                                nc.vector.tensor_mul(
                                    outer,
                                    xkreps[j][:, ph * HCOL:(ph + 1) * HCOL],
                                    x0bcs[g][:, ph * HCOL:(ph + 1) * HCOL],
                                )
                            mm_list.append((k, outer))
                        for ql in range(NG // 2):
                            qb = ph * (NG // 2) + ql
                            for n, (k, outer) in enumerate(mm_list):
                                nc.tensor.matmul(
                                    psums[qb],
                                    lhsT=w3_sb[:, k * C:(k + 1) * C],
                                    rhs=outer[:, ql * GW:(ql + 1) * GW],
                                    start=(k == 0),
                                    stop=(last_grp and n == len(mm_list) - 1),
                                )
                    assert not pending
                    # bias add into paired-bank SBUF tiles; one store per
                    # bank pair (2KB-per-partition descriptors, half the
                    # store-chain overhead of per-bank stores)
                    bpg = BC // NG
                    for pair in range(2):
                        out_sb = opool.tile(
                            [128, 2 * GW], f32, name=f"osb{ph}_{pair}",
                            tag=f"osb{pair}",
                        )
                        for half in range(2):
                            ql = pair * 2 + half
                            qb = ph * (NG // 2) + ql
                            nc.scalar.activation(
                                out_sb[:, half * GW:(half + 1) * GW],
                                psums[qb],
                                mybir.ActivationFunctionType.Identity,
                                bias=bias_sb[:, 0:1],
                                scale=1.0,
                            )
                        qb0 = ph * (NG // 2) + pair * 2
                        nc.sync.dma_start(
                            out=out_d.ap()[:, qb0 * bpg:(qb0 + 2) * bpg, :],
                            in_=out_sb,
                        )

    nc.compile()
    return nc


def _prep_host(x0, xk, W, b):
    """Host-side layout prep (no arithmetic): shard, transpose, replicate."""
    part = np.arange(128)
    hh = (part % HB)[None, :] + HB * np.arange(NHB)[:, None]   # [NHB, 128]
    mm = (part // HB)[None, :] + MG * np.arange(NMG)[:, None]  # [NMG, 128]

    Wr = W.reshape(C, H, M)
    w3 = np.empty((128, NCHUNK, C), ml_dtypes.bfloat16)
    for g in range(NMG):
        for j in range(NHB):
            w3[:, g * NHB + j, :] = Wr[:, hh[j], mm[g]].T.astype(
                ml_dtypes.bfloat16
            )
    w3 = np.ascontiguousarray(w3.reshape(128, NCHUNK * C))
    bias = np.ascontiguousarray(b.reshape(C, 1)).astype(np.float32)

    in_maps = []
    for k in range(N_CORES):
        x0s = x0[k * BC:(k + 1) * BC]            # [BC, M, D]
        xks = xk[k * BC:(k + 1) * BC]            # [BC, H, D]
        xk2 = (
            np.ascontiguousarray(xks.transpose(1, 0, 2))
            .reshape(H, COLS)
            .astype(ml_dtypes.bfloat16)
        )
        x02 = (
            np.ascontiguousarray(x0s.transpose(1, 0, 2))
            .reshape(M, COLS)
            .astype(ml_dtypes.bfloat16)
        )
        in_maps.append(
            {
                "xkrep_in": np.ascontiguousarray(xk2[hh]),
                "x0bc_in": np.ascontiguousarray(x02[mm]),
                "w3_in": w3,
                "bias_in": bias,
            }
        )
    return in_maps


def _run(in_maps, **kwargs):
    from concourse import bass_utils

    if "nc" not in _cache:
        _cache["nc"] = _build()
    return bass_utils.run_bass_kernel_spmd(
        _cache["nc"], in_maps, core_ids=list(range(N_CORES)), **kwargs
    )


def kernel(x0, xk, W, b, _bench=[None]):
    x0 = np.asarray(x0, dtype=np.float32)
    xk = np.asarray(xk, dtype=np.float32)
    W = np.asarray(W, dtype=np.float32)
    b = np.asarray(b, dtype=np.float32)
    in_maps = _prep_host(x0, xk, W, b)
    res = _run(in_maps)
    _bench[0] = res
    # per-core out is c-major [C, BC, D]; restore [BC, C, D] and stack cores
    out = np.concatenate(
        [np.transpose(r["out"], (1, 0, 2)) for r in res.results], axis=0
    )
    return np.ascontiguousarray(out, dtype=np.float32)
